# revision 7
# baseline (speedup 1.0000x reference)
"""Trainium2 Bass kernel for nn_ASSC_66657892434080 (v2).

Reference computation (per batch sample b, data-parallel over 8 cores):
    q = wq @ x_1[b] + bq ; k = wk @ x[b] + bk          (1x1 convs)
    proj_query = PSP(q) [256,280] ; proj_key = PSP(k) [32,280]
    aff = sigmoid(proj_query @ proj_key^T)             [256,32]
    out[b] = conv3x3(x_1[b], (aff @ con.reshape(32,-1)))   (grouped conv)

v2 restructuring vs the 143us baseline (engine-bottleneck driven):
  * x_1/x loaded as CONTIGUOUS [128, 9216] tiles (18KB descriptors, full DMA
    speed) instead of zero-padded [128,98,98] tiles (192B descriptors, 2x
    latency penalty, ~26us of DMA-engine time).
  * conv borders via partial-extent matmuls (dx=+-1 use 95-wide column
    windows) instead of a padded tile; dx=0 (full width) carries start=True.
  * zs PSUM->SBUF copies: ONE [96,384] copy per chunk (Act) + 3 SBUF->SBUF
    shift DMAs at the end apply the per-dy row shift (was 3 narrow copies
    per chunk, ~33us of Act time -> ~11us + 2us DMA).
  * pooling scratch in f16 (validated 3.8e-3 end-to-end): halves SBUF.
  * prefix scans (DVE-only op, 2.11ns/elem) stay on DVE; ALL bin-diff ops
    move to GPSIMD (idle otherwise); q-side H-diffs on DVE (GPSIMD is busy
    with the 58 k-side clipped-bin diffs then).
  * k-proj loops ch-major so the first 9 column-chunks run as soon as the
    first halves of x land.
"""

import numpy as np
import concourse.bass as bass
import concourse.bacc as bacc
import concourse.tile as tile
import concourse.mybir as mybir
import bass_rust
from concourse.bass_utils import run_bass_kernel_spmd

B, C, H, W = 8, 256, 96, 96
C8 = 32
HW = H * W                      # 9216
POOL_SIZES = (1, 3, 5, 7, 14)   # -> 30 1-D bins, 280 2-D positions
NB = sum(POOL_SIZES)            # 30
NP = sum(s * s for s in POOL_SIZES)  # 280
TH = 32                         # k-side rows per third
ROWS = 4                        # conv rows per PSUM chunk
NCH = H // ROWS                 # 24 chunks
HHW = HW // 2                   # 4608
F32 = mybir.dt.float32
BF16 = mybir.dt.bfloat16
F16 = mybir.dt.float16

# smalls packing (f32, [128, 696]): bq | bk | ai | ai3(3x32) | id32
SM_BQ, SM_BK, SM_AI, SM_AI3, SM_ID, SM_N = 0, 256, 288, 568, 664, 696


def _pool_bins(n, s):
    return [((i * n) // s, -((-(i + 1) * n) // s)) for i in range(s)]


HBINS = {s: _pool_bins(H, s) for s in POOL_SIZES}
JBASE = {}
B280 = {}
_j = _p = 0
for _s in POOL_SIZES:
    JBASE[_s] = _j
    B280[_s] = _p
    _j += _s
    _p += _s * _s


def _area_inv():
    ai = np.zeros(NP, np.float32)
    for s in POOL_SIZES:
        hb, wb = _pool_bins(H, s), _pool_bins(W, s)
        for o, (hs, he) in enumerate(hb):
            for p, (ws, we) in enumerate(wb):
                ai[B280[s] + o * s + p] = 1.0 / ((he - hs) * (we - ws))
    return ai


def _split_multiwait_ctrl(nc, default_limit=1):
    """walrus in this container rejects instructions carrying more than one
    sem wait; move extras onto preceding same-engine drains.  NEVER split PE
    instructions (reorder window pulls LDWEIGHTS ahead)."""
    for f in nc.m.functions:
        for bb in f.blocks:
            new_list = []
            for inst in bb.instructions:
                si = inst.sync_info
                waits = list(si.on_wait) if si and si.on_wait else []
                mw = default_limit
                if getattr(inst, "engine", None) == mybir.EngineType.PE:
                    mw = 99
                if len(waits) > mw:
                    for k, w in enumerate(waits[:-mw]):
                        pre = mybir.InstDrain(name=f"{inst.name}-w{k}", ins=[], outs=[])
                        pre.engine = inst.engine
                        pre.sync_info = bass_rust.SyncInfo(on_wait=[w], on_update=[])
                        new_list.append(pre)
                    inst.sync_info = bass_rust.SyncInfo(
                        on_wait=waits[-mw:],
                        on_update=list(si.on_update) if si.on_update else [],
                    )
                new_list.append(inst)
            bb.instructions[:] = new_list


def _sv(ap2d, dims):
    """Strided view: keep partition dim, replace free dims with (step, count)."""
    import dataclasses
    return dataclasses.replace(ap2d, ap=[list(ap2d.ap[0])] + [[s, c] for s, c in dims])


def _uniform_runs(bins):
    """Group consecutive bins into runs with constant boundary strides."""
    runs = []
    i = 0
    n = len(bins)
    while i < n:
        if i == n - 1:
            runs.append((i, 1, 0, 0))
            i += 1
            continue
        ds = bins[i + 1][0] - bins[i][0]
        de = bins[i + 1][1] - bins[i][1]
        j = i + 1
        while (j + 1 < n and bins[j + 1][0] - bins[j][0] == ds
               and bins[j + 1][1] - bins[j][1] == de):
            j += 1
        runs.append((i, j - i + 1, ds, de))
        i = j + 1
    return runs


def _fix_ldweights_waits(nc):
    """Move waits that gate weight data from InstMatmult to its InstLdweights
    (prevents stale-weight races after Tile's 2-byte matmul split)."""
    import copy
    for f in nc.m.functions:
        for bb in f.blocks:
            insts = bb.instructions
            new_list = []
            i = 0
            while i < len(insts):
                inst = insts[i]
                nxt = insts[i + 1] if i + 1 < len(insts) else None
                if (type(inst).__name__ == "InstLdweights" and nxt is not None
                        and type(nxt).__name__ == "InstMatmult"):
                    wl = list(inst.sync_info.on_wait) if inst.sync_info and inst.sync_info.on_wait else []
                    wm = list(nxt.sync_info.on_wait) if nxt.sync_info and nxt.sync_info.on_wait else []
                    waits = wl + wm
                    mm_upd = list(nxt.sync_info.on_update) if nxt.sync_info and nxt.sync_info.on_update else []
                    ld_upd = list(inst.sync_info.on_update) if inst.sync_info and inst.sync_info.on_update else []
                    if len(waits) > 1:
                        for k, w in enumerate(waits[:-1]):
                            pre = copy.deepcopy(inst)
                            pre.name = f"{inst.name}-ldw{k}"
                            pre.sync_info = bass_rust.SyncInfo(on_wait=[w], on_update=[])
                            new_list.append(pre)
                        inst.sync_info = bass_rust.SyncInfo(on_wait=[waits[-1]], on_update=ld_upd)
                        nxt.sync_info = bass_rust.SyncInfo(on_wait=[], on_update=mm_upd)
                    elif len(waits) == 1:
                        inst.sync_info = bass_rust.SyncInfo(on_wait=[waits[0]], on_update=ld_upd)
                        nxt.sync_info = bass_rust.SyncInfo(on_wait=[], on_update=mm_upd)
                    new_list.append(inst)
                    new_list.append(nxt)
                    i += 2
                    continue
                new_list.append(inst)
                i += 1
            bb.instructions[:] = new_list


def build_kernel(split_ctrl=True):
    nc = bacc.Bacc("TRN2", target_bir_lowering=False, debug=False)
    add, byp = mybir.AluOpType.add, mybir.AluOpType.bypass

    x1 = nc.dram_tensor("x1", [2, 128, HW], BF16, kind="ExternalInput")
    xx = nc.dram_tensor("xx", [2, 128, HW], BF16, kind="ExternalInput")
    wqT = nc.dram_tensor("wqT", [2, 128, C], F32, kind="ExternalInput")
    wkTb = nc.dram_tensor("wkTb", [2, 128, C8], BF16, kind="ExternalInput")
    conz = nc.dram_tensor("conz", [6, 128, 96], BF16, kind="ExternalInput")
    smalls = nc.dram_tensor("smalls", [128, SM_N], F32, kind="ExternalInput")
    out = nc.dram_tensor("out", [2, 128, HW], BF16, kind="ExternalOutput")

    with tile.TileContext(nc) as tc:
        with (
            tc.tile_pool(name="consts", bufs=1) as cpool,
            tc.tile_pool(name="xpool", bufs=1) as xpool,
            tc.tile_pool(name="scratch", bufs=1) as spool,
            tc.tile_pool(name="ostage", bufs=2) as opool,
        ):
            # ---- tiles ----
            czt = cpool.tile([128, 576], BF16, tag="czt", name="czt")
            wkt = cpool.tile([128, 2 * C8], BF16, tag="wkt", name="wkt")
            wq_t = [cpool.tile([128, C], F32, tag=f"wq{i}", name=f"wq{i}") for i in range(2)]
            smt = cpool.tile([128, SM_N], F32, tag="smt", name="smt")
            x1t = [xpool.tile([128, HW], BF16, tag=f"x1t{i}", name=f"x1t{i}") for i in range(2)]
            xxt = [xpool.tile([128, HW], BF16, tag=f"xxt{i}", name=f"xxt{i}") for i in range(2)]

            F1s = [spool.tile([128, HHW + 1], F16, tag=f"F1{i}", name=f"F1{i}") for i in range(2)]
            Gq = [spool.tile([128, NB * H], F16, tag=f"G{i}", name=f"G{i}") for i in range(2)]
            F2q = [spool.tile([128, NB * H + 1], F16, tag=f"F2{i}", name=f"F2{i}") for i in range(2)]
            F1k = spool.tile([96, TH * W + 1], F16, tag="F1k", name="F1k")
            Gk = spool.tile([96, NB * TH], F16, tag="Gk", name="Gk")
            F2k = spool.tile([96, NB * TH + 1], F16, tag="F2k", name="F2k")
            kq = spool.tile([96, TH * W], F16, tag="kq", name="kq")
            zs_raw = spool.tile([96, HW], BF16, tag="zsr", name="zsr")
            zss = spool.tile([96, HW], BF16, tag="zss", name="zss")
            Pq = [spool.tile([128, NP], F32, tag=f"Pq{i}", name=f"Pq{i}") for i in range(2)]
            Dk = spool.tile([96, NP], F32, tag="Dk", name="Dk")
            Dsh = [spool.tile([32, NP], F32, tag=f"Dsh{t}", name=f"Dsh{t}") for t in range(2)]

            # ---- input DMAs (SP queue order == issue order) ----
            nc.sync.dma_start(czt[:], _sv(conz.ap()[0], [(128 * 96, 6), (1, 96)]))
            nc.sync.dma_start(wkt[:], _sv(wkTb.ap()[0], [(128 * C8, 2), (1, C8)]))
            nc.sync.dma_start(x1t[0][:, :HHW], x1.ap()[0][:, :HHW])
            nc.sync.dma_start(x1t[1][:, :HHW], x1.ap()[1][:, :HHW])
            nc.sync.dma_start(xxt[0][:, :HHW], xx.ap()[0][:, :HHW])
            nc.sync.dma_start(x1t[0][:, HHW:], x1.ap()[0][:, HHW:])
            nc.sync.dma_start(xxt[1][:, :HHW], xx.ap()[1][:, :HHW])
            nc.sync.dma_start(x1t[1][:, HHW:], x1.ap()[1][:, HHW:])
            nc.sync.dma_start(xxt[0][:, HHW:], xx.ap()[0][:, HHW:])
            nc.sync.dma_start(xxt[1][:, HHW:], xx.ap()[1][:, HHW:])
            for i in range(2):
                nc.sync.dma_start(wq_t[i][:], wqT.ap()[i])
            nc.sync.dma_start(smt[:], smalls.ap())

            bq_t = smt[:, SM_BQ:SM_BQ + C]
            bk_t = smt[:, SM_BK:SM_BK + C8]
            ai_t = smt[:, SM_AI:SM_AI + NP]
            id32 = smt[:, SM_ID:SM_ID + 32]

            # ---- DVE warmup + scratch init (runs while first DMAs land) ----
            nc.vector.memset(F1s[0][:, :2048], 0.0)
            nc.vector.memset(F1s[1][:, :2048], 0.0)
            for t_ in F2q:
                nc.vector.memset(t_[:, 0:1], 0.0)
            nc.vector.memset(F1k[:96, 0:1], 0.0)
            nc.vector.memset(F2k[:96, 0:1], 0.0)
            nc.gpsimd.memset(Dk[:, :], 0.0)
            nc.gpsimd.memset(zss[0:32, 0:W], 0.0)
            nc.gpsimd.memset(zss[64:96, (H - 1) * W:HW], 0.0)

            # ---- zconv: chunks with border-split partial matmuls ----
            # conz layout: czt[:, (dx*2+cinc)*96 : +96], dx tap = x[w+dx-1]
            def zconv_chunk(pzp, j):
                r0 = j * ROWS
                zp = pzp.tile([96, ROWS * W], F32, tag="zp", name="zp")
                base = r0 * W
                # dx=1 (center, full width) first: carries start=True
                for cinc in range(2):
                    nc.tensor.matmul(
                        zp[:, :], czt[:, (2 + cinc) * 96:(3 + cinc) * 96],
                        _sv(x1t[cinc][:, base:], [(W, ROWS), (1, W)]),
                        start=(cinc == 0), stop=False)
                # dx=0 (left tap): out cols 1..95 <- x cols 0..94
                for cinc in range(2):
                    nc.tensor.matmul(
                        _sv(zp[:, 1:], [(W, ROWS), (1, W - 1)]),
                        czt[:, (0 + cinc) * 96:(1 + cinc) * 96],
                        _sv(x1t[cinc][:, base:], [(W, ROWS), (1, W - 1)]),
                        start=False, stop=False)
                # dx=2 (right tap): out cols 0..94 <- x cols 1..95
                for cinc in range(2):
                    nc.tensor.matmul(
                        _sv(zp[:, 0:], [(W, ROWS), (1, W - 1)]),
                        czt[:, (4 + cinc) * 96:(5 + cinc) * 96],
                        _sv(x1t[cinc][:, base + 1:], [(W, ROWS), (1, W - 1)]),
                        start=False, stop=(cinc == 1))
                nc.scalar.copy(zs_raw[:, base:base + ROWS * W], zp[:, :])

            with tc.tile_pool(name="pz", bufs=3, space="PSUM") as pzp, \
                 tc.tile_pool(name="pk", bufs=2, space="PSUM") as pkp:
                for j in range(NCH // 2):
                    zconv_chunk(pzp, j)
                # k-proj ch-major: ch 0..8 need only first halves of xx
                for ch in range(HW // 512):
                    t = ch // 6
                    kp = pkp.tile([128, 512], F32, tag="kp", name="kp")
                    for cc in range(2):
                        nc.tensor.matmul(kp[32 * t:32 * t + 32, :],
                                         wkt[:, cc * C8:(cc + 1) * C8],
                                         xxt[cc][:, ch * 512:(ch + 1) * 512],
                                         start=(cc == 0), stop=(cc == 1),
                                         tile_position=(0, 32 * t))
                    nc.scalar.copy(kq[32 * t:32 * t + 32,
                                      (ch % 6) * 512:(ch % 6 + 1) * 512],
                                   kp[32 * t:32 * t + 32, :])
                for j in range(NCH // 2, NCH):
                    zconv_chunk(pzp, j)

            # ---- pooling: scans on DVE, bin-diffs on GPSIMD/DVE ----
            def w_diffs(eng, F1, G, h0, rows):
                jbase = 0
                for s in POOL_SIZES:
                    sb = _pool_bins(W, s)
                    for (i0, cnt, ds, de) in _uniform_runs(sb):
                        ws, we = sb[i0]
                        j0 = jbase + i0
                        eng.tensor_tensor(
                            _sv(G[:, j0 * H + h0:], [(H, cnt), (1, rows)]),
                            _sv(F1[:, we:], [(de, cnt), (W, rows)]),
                            _sv(F1[:, ws:], [(ds, cnt), (W, rows)]),
                            mybir.AluOpType.subtract)
                    jbase += s

            def h_diffs(eng, F2, P_out):
                for s in POOL_SIZES:
                    jb = JBASE[s]
                    hb = HBINS[s]
                    for (o0, cnt, ds, de) in _uniform_runs(hb):
                        hs, he = hb[o0]
                        eng.tensor_tensor(
                            _sv(P_out[:, B280[s] + o0 * s:], [(s, cnt), (1, s)]),
                            _sv(F2[:, jb * H + he:], [(de, cnt), (H, s)]),
                            _sv(F2[:, jb * H + hs:], [(ds, cnt), (H, s)]),
                            mybir.AluOpType.subtract)

            # q scans (DVE) + W-diffs (GPSIMD)
            for cc in range(2):
                for strip in range(2):
                    F1 = F1s[strip]
                    nc.vector.tensor_tensor_scan(
                        F1[:, 1:1 + HHW], x1t[cc][:, strip * HHW:(strip + 1) * HHW],
                        _sv(F1[:, 0:1], [(0, HHW)]), 0.0, add, byp)
                    w_diffs(nc.gpsimd, F1, Gq[cc], strip * 48, 48)

            # k W-scan + diffs
            nc.vector.tensor_tensor_scan(
                F1k[:96, 1:1 + TH * W], kq[:, :],
                _sv(F1k[:96, 0:1], [(0, TH * W)]), 0.0, add, byp)
            # k W-diffs into Gk [96, NB*TH] flat j*32+h (GPSIMD)
            jbase = 0
            for s in POOL_SIZES:
                sb = _pool_bins(W, s)
                for (i0, cnt, ds, de) in _uniform_runs(sb):
                    ws, we = sb[i0]
                    j0 = jbase + i0
                    nc.gpsimd.tensor_tensor(
                        _sv(Gk[:96, j0 * TH:], [(TH, cnt), (1, TH)]),
                        _sv(F1k[:96, we:], [(de, cnt), (W, TH)]),
                        _sv(F1k[:96, ws:], [(ds, cnt), (W, TH)]),
                        mybir.AluOpType.subtract)
                jbase += s
            # k F2 scan (DVE)
            nc.vector.tensor_tensor_scan(
                F2k[:96, 1:1 + NB * TH], Gk[:96, :],
                _sv(F2k[:96, 0:1], [(0, NB * TH)]), 0.0, add, byp)
            # k H-diffs: third-local clipped bins, one instr per bin (GPSIMD)
            for t in range(3):
                for s in POOL_SIZES:
                    jb = JBASE[s]
                    for o, (hs, he) in enumerate(HBINS[s]):
                        lhs = min(max(hs - TH * t, 0), TH)
                        lhe = min(max(he - TH * t, 0), TH)
                        if lhe <= lhs:
                            continue
                        nc.gpsimd.tensor_tensor(
                            _sv(Dk[32 * t:32 * t + 32, B280[s] + o * s:], [(1, s)]),
                            _sv(F2k[32 * t:32 * t + 32, jb * TH + lhe:], [(TH, s)]),
                            _sv(F2k[32 * t:32 * t + 32, jb * TH + lhs:], [(TH, s)]),
                            mybir.AluOpType.subtract)

            # q F2 scans + H-diffs (diffs on DVE; GPSIMD busy with k H-diffs)
            for cc in range(2):
                nc.vector.tensor_tensor_scan(
                    F2q[cc][:, 1:1 + NB * H], Gq[cc][:, :],
                    _sv(F2q[cc][:, 0:1], [(0, NB * H)]), 0.0, add, byp)
                h_diffs(nc.vector, F2q[cc], Pq[cc])
                nc.vector.tensor_tensor(Pq[cc][:], Pq[cc][:], ai_t,
                                        mybir.AluOpType.mult)

            # partition-shift thirds 1,2 of Dk down to base 0 for matmuls
            for t in range(2):
                nc.sync.dma_start(Dsh[t][:, :], Dk[32 * (t + 1):32 * (t + 2), :])

            # zs shift DMAs (apply per-dy row shift) + out staging
            nc.sync.dma_start(zss[0:32, W:HW], zs_raw[0:32, 0:HW - W])
            nc.sync.dma_start(zss[32:64, :], zs_raw[32:64, :])
            nc.sync.dma_start(zss[64:96, 0:HW - W], zs_raw[64:96, W:HW])

            # ---- projections / affinity ----
            PCH = [(0, 128), (128, 128), (256, 24)]
            pqT = [cpool.tile([n, C], F32, tag=f"pqT{i}", name=f"pqT{i}")
                   for i, (_, n) in enumerate(PCH)]
            pkR = [cpool.tile([n, 96], F32, tag=f"pkR{i}", name=f"pkR{i}")
                   for i, (_, n) in enumerate(PCH)]
            mk1 = cpool.tile([128, C8], F32, tag="mk1", name="mk1")
            affT96 = cpool.tile([96, C], BF16, tag="affT96", name="affT96")

            with tc.tile_pool(name="psmall", bufs=2, space="PSUM") as pps:
                for i, (p0, n) in enumerate(PCH):
                    ps2 = pps.tile([n, C8], F32, tag="ps2", name="ps2")
                    dsrc = [Dk, Dsh[0], Dsh[1]]
                    for t in range(3):
                        nc.tensor.matmul(ps2[:], dsrc[t][0:32, p0:p0 + n],
                                         id32[0:32, :],
                                         start=(t == 0), stop=(t == 2))
                    nc.vector.tensor_tensor(mk1[:n, :], ps2[:],
                                            smt[:n, SM_AI3 + 32 * i:SM_AI3 + 32 * (i + 1)],
                                            mybir.AluOpType.mult)
                    for g in range(3):
                        nc.gpsimd.tensor_tensor(pkR[i][:, 32 * g:32 * g + 32],
                                                mk1[:n, :], bk_t[:n, :],
                                                mybir.AluOpType.add)
                for i, (p0, n) in enumerate(PCH):
                    ps = pps.tile([n, C], F32, tag="ps", name="ps")
                    for cc in range(2):
                        nc.tensor.matmul(ps[:], Pq[cc][:, p0:p0 + n], wq_t[cc][:],
                                         start=(cc == 0), stop=(cc == 1))
                    nc.vector.tensor_tensor(pqT[i][:], ps[:], bq_t[:n, :],
                                            mybir.AluOpType.add)
                pa = pps.tile([96, C], F32, tag="pa", name="pa")
                for i in range(3):
                    nc.tensor.matmul(pa[:], pkR[i][:], pqT[i][:],
                                     start=(i == 0), stop=(i == 2))
                nc.scalar.activation(affT96[:], pa[:], mybir.ActivationFunctionType.Sigmoid)

            # ---- out matmuls: 24 chunks x 2 cout halves; GB=6 staging ----
            GB = 6
            with tc.tile_pool(name="po", bufs=4, space="PSUM") as pop:
                ot = [None, None]
                for j in range(NCH):
                    r0 = j * ROWS
                    for coutc in range(2):
                        op = pop.tile([128, ROWS * W], F32, tag="op", name="op")
                        nc.tensor.matmul(op[:], affT96[:, coutc * 128:(coutc + 1) * 128],
                                         zss[:, r0 * W:(r0 + ROWS) * W],
                                         start=True, stop=True)
                        if j % GB == 0:
                            ot[coutc] = opool.tile([128, GB * ROWS * W], BF16,
                                                   tag=f"ot{coutc}", name=f"ot{coutc}")
                        seg = (j % GB) * ROWS * W
                        if (j + coutc) % 2 == 0:
                            nc.scalar.copy(ot[coutc][:, seg:seg + ROWS * W], op[:])
                        else:
                            nc.vector.tensor_copy(ot[coutc][:, seg:seg + ROWS * W], op[:])
                        if j % GB == GB - 1:
                            nc.sync.dma_start(
                                out.ap()[coutc][:, (r0 - (GB - 1) * ROWS) * W:(r0 + ROWS) * W],
                                ot[coutc][:])

    if split_ctrl:
        nc.compile()
        _fix_ldweights_waits(nc)
    return nc


_NC_CACHE = {}


def _get_nc():
    if "nc" not in _NC_CACHE:
        _NC_CACHE["nc"] = build_kernel()
    return _NC_CACHE["nc"]


def _conv_cast(x):
    import ml_dtypes
    return np.ascontiguousarray(x, np.float32).astype(ml_dtypes.bfloat16)


def kernel(x_1, x, wq, bq, wk, bk, con):
    import ml_dtypes
    x_1 = _conv_cast(x_1)
    x = _conv_cast(x)
    con = np.asarray(con, np.float32)
    wq = np.asarray(wq, np.float32)
    bq = np.asarray(bq, np.float32)
    wk = np.asarray(wk, np.float32)
    bk = np.asarray(bk, np.float32)

    wqT_h = np.ascontiguousarray(wq.T).reshape(2, 128, C)
    wkTb_h = np.ascontiguousarray(wk.T).reshape(2, 128, C8).astype(ml_dtypes.bfloat16)
    # conz[dx*2+cinc, cin, dy*32+kk] = con[kk, cinc*128+cin, dy, dx]
    conz_h = np.ascontiguousarray(
        con.transpose(3, 1, 2, 0)          # [dx, cin256, dy, kk]
        .reshape(3, 2, 128, 3 * C8)
        .reshape(6, 128, 96)
    ).astype(ml_dtypes.bfloat16)
    ai = _area_inv()
    smalls_h = np.zeros((128, SM_N), np.float32)
    smalls_h[:, SM_BQ:SM_BQ + C] = bq[None, :]
    smalls_h[:, SM_BK:SM_BK + C8] = bk[None, :]
    smalls_h[:, SM_AI:SM_AI + NP] = ai[None, :]
    for i, (p0, n) in enumerate([(0, 128), (128, 128), (256, 24)]):
        smalls_h[:n, SM_AI3 + 32 * i:SM_AI3 + 32 * (i + 1)] = ai[p0:p0 + n, None]
    smalls_h[:32, SM_ID:SM_ID + 32] = np.eye(32, dtype=np.float32)

    in_maps = []
    for b in range(B):
        in_maps.append({
            "x1": x_1[b].reshape(2, 128, HW),
            "xx": x[b].reshape(2, 128, HW),
            "wqT": wqT_h, "wkTb": wkTb_h, "conz": conz_h, "smalls": smalls_h,
        })
    global _last_in_maps
    _last_in_maps = in_maps
    nc = _get_nc()
    res = run_bass_kernel_spmd(nc, in_maps, list(range(B)))
    return np.stack([res.results[b]["out"].astype(np.float32).reshape(C, H, W)
                     for b in range(B)])
